# revision 28
# baseline (speedup 1.0000x reference)
"""Multi-head causal self-attention (B=2, T=2048, C=768, H=12, D=64) on 8
Trainium2 NeuronCores.

Sharding: 24 (batch, head) units -> 3 heads per core; cores 0-3 take batch 0,
cores 4-7 take batch 1. Each core computes q/k/v projections for its 3 heads,
flash-style causal attention fully on-chip (no T x T tensor ever touches HBM),
and a partial output projection with its 192-row slice of Wproj. The host sums
the 4 partial projections per batch.

Device design notes:
  - All matmuls bf16 with fp32 PSUM accumulation.
  - DRAM tensors are packed so every DMA row is 1.5-6KB contiguous (the DMA
    queue is packet-rate limited, so big packets are the difference between
    ~65 GB/s and ~350 GB/s effective). Weights stream on the Scalar engine's
    DMA queue, x and outputs on Sync's, so descriptor feeds run in parallel.
  - q^T/k^T live in [64, T]-per-head bf16 tiles; S^T = K^T.T @ Q^T is
    computed transposed [tk, tq] so exp(S^T) feeds the P.T @ V matmul
    directly - no on-chip transposes anywhere. K^T tiles are zero-padded to
    K=128 and the zero-block position selects which half of the shared
    [q0;q1] rhs tile contributes.
  - V is augmented with a ones column per head, so the PV accumulation
    yields the softmax denominator as psum row 64 for free.
  - Normalization: fast approx reciprocal of the denominator row ([1, TQ] on
    DVE straight from PSUM), partition-broadcast to 64 rows on GpSimd, one
    DVE multiply. No PE involvement.
  - Causal masking: matmul columns restricted to tq >= tk-block start; the
    diagonal 128x128 sub-block gets a strictly-lower-triangular zero mask on
    P^T (GpSimd affine_select) after exp. Chunk-major emission pipelines QKV
    production, attention, and the output projection.
  - Output is written per chunk-half in [128, 1536] layout (3KB rows);
    host reassembles and reduces.
"""

import os
import sys

sys.path.insert(0, "/opt/trn_rl_repo")

import ml_dtypes
import numpy as np

import concourse.bass as bass
import concourse.tile as tile
from concourse import bacc, mybir
from concourse import bass_utils

B, T, C = 2, 2048, 768
H, D = 12, 64
N_CORES = 8
H_LOC = 3           # heads per core
DL = H_LOC * D      # 192 local head dims
TQ = 512            # tq chunk (psum bank width)
TB = 128            # tk block
NCH = T // TQ       # 4 chunks
NBL = TQ // TB      # 4 blocks per chunk
NKT = C // 128      # 6 contraction k-tiles
VW = 196            # v psum width: 3*(D+1)=195 used + 1 pad
XW = NKT * TQ       # 3072 x cols per chunk
XH = XW // 2        # 1536 half-chunk cols

f32 = mybir.dt.float32
bf16 = mybir.dt.bfloat16
EXP = mybir.ActivationFunctionType.Exp
SIMSAFE = bool(os.environ.get("BASS_SIMSAFE"))  # zero psum holes for CoreSim

LAST_RESULT = None  # test harness reads exec_time_ns from here


def _build_program(use_bias: bool):
    from contextlib import ExitStack

    nc = bacc.Bacc("TRN2", target_bir_lowering=False, debug=False,
                   num_devices=N_CORES)

    xt_d = nc.dram_tensor("xt", [NCH, 128, XW], bf16, kind="ExternalInput").ap()
    xt1_d = nc.dram_tensor("xt1", [1, TQ], bf16, kind="ExternalInput").ap()
    wqk_d = nc.dram_tensor("wqk", [128, 7 * 2 * DL], bf16, kind="ExternalInput").ap()
    wv_d = nc.dram_tensor("wv", [128, 7 * VW], bf16, kind="ExternalInput").ap()
    wp_d = nc.dram_tensor("wp", [128, 2 * C], bf16, kind="ExternalInput").ap()
    out_d = nc.dram_tensor("outc", [NCH, 128, NKT * TQ], bf16,
                           kind="ExternalOutput").ap()

    with tile.TileContext(nc) as tc, ExitStack() as ctx:
        wpool = ctx.enter_context(tc.tile_pool(name="w", bufs=1))
        xpool = ctx.enter_context(tc.tile_pool(name="x", bufs=1))
        qkpool = ctx.enter_context(tc.tile_pool(name="qk", bufs=1))
        cpool = ctx.enter_context(tc.tile_pool(name="const", bufs=1))
        ones_b = cpool.tile([1, D], bf16)
        nc.vector.memset(ones_b[:], 1.0)

        # --- input loads. Weights go on the Scalar engine's DMA queue, x on
        # Sync's: two descriptor feeds in parallel, all rows >= 1.5KB.
        # The first qk matmul needs only wqk k-tile 0 and x chunk-0 k-tile 0,
        # so those get their own small tiles/DMAs at the head of each queue;
        # the rest stream in bigger transfers behind them.
        weng = nc.scalar
        wqk0 = wpool.tile([128, 2 * DL], bf16, tag="wqk0", name="wqk0")
        weng.dma_start(wqk0[:], wqk_d[:, 0 : 2 * DL])
        wqk12 = wpool.tile([128, 2 * 2 * DL], bf16, tag="wqk12", name="wqk12")
        weng.dma_start(wqk12[:], wqk_d[:, 2 * DL : 3 * 2 * DL])
        wqkB = wpool.tile([128, 4 * 2 * DL], bf16, tag="wqkB", name="wqkB")
        weng.dma_start(wqkB[:], wqk_d[:, 3 * 2 * DL : 7 * 2 * DL])
        wv = wpool.tile([128, 7 * VW], bf16, tag="wv", name="wv")
        weng.dma_start(wv[:], wv_d[:])
        wp = wpool.tile([128, 2 * C], bf16, tag="wp", name="wp")
        weng.dma_start(wp[:], wp_d[:])
        if use_bias:
            xt1 = xpool.tile([1, TQ], bf16)
            weng.dma_start(xt1[:], xt1_d[:])

        xt0 = []
        for part, w in ((0, 1), (1, 2), (3, 3)):
            t_ = xpool.tile([128, TQ * w], bf16, tag=f"xt0_{part}",
                            name=f"xt0_{part}")
            nc.sync.dma_start(t_[:], xt_d[0][:, TQ * part : TQ * (part + w)])
            xt0.append(t_)
        xh = [None]
        for t in range(1, NCH):
            halves = []
            for hf in range(2):
                t_ = xpool.tile([128, XH], bf16, tag=f"xt{t}_{hf}",
                                name=f"xt{t}_{hf}")
                nc.sync.dma_start(t_[:], xt_d[t][:, XH * hf : XH * (hf + 1)])
                halves.append(t_)
            xh.append(halves)

        def xtile(t, j):
            if t == 0:
                part = 0 if j == 0 else (1 if j < 3 else 2)
                base = (0, 1, 3)[part]
                return xt0[part][:, TQ * (j - base) : TQ * (j - base + 1)]
            return xh[t][j // 3][:, TQ * (j % 3) : TQ * (j % 3 + 1)]

        def wqk_sl(j, m, rows=None):
            if j == 0:
                tl, base = wqk0, 0
            elif j < 3:
                tl, base = wqk12, (j - 1) * 2 * DL
            else:
                tl, base = wqkB, (j - 3) * 2 * DL
            sl = tl[:, base + 128 * m : base + 128 * (m + 1)]
            return sl if rows is None else tl[0:rows, base + 128 * m : base + 128 * (m + 1)]

        # Attention-stage tiles (bf16). S^T contraction is zero-padded to
        # K=128; each head's K^T has the other 64 rows zeroed, and the
        # zero-block position selects which half of the shared [q0;q1] rhs
        # tile contributes.
        qTA, qTC, kT0, kT1, kT2 = [], [], [], [], []
        for t in range(NCH):
            qTA.append(qkpool.tile([128, TQ], bf16, tag=f"qTA{t}", name=f"qTA{t}"))  # [q0 ; q1]
            qTC.append(qkpool.tile([128, TQ], bf16, tag=f"qTC{t}", name=f"qTC{t}"))  # [q2 ; *]
            kT0.append(qkpool.tile([128, TQ], bf16, tag=f"kT0{t}", name=f"kT0{t}"))  # [k0 ; 0]
            kT1.append(qkpool.tile([128, TQ], bf16, tag=f"kT1{t}", name=f"kT1{t}"))  # [0 ; k1]
            kT2.append(qkpool.tile([128, TQ], bf16, tag=f"kT2{t}", name=f"kT2{t}"))  # [k2 ; 0]
            nc.gpsimd.memset(kT0[t][64:128, :], 0.0)
            nc.gpsimd.memset(kT1[t][0:64, :], 0.0)
            nc.gpsimd.memset(kT2[t][64:128, :], 0.0)
            # qTC rows 64+ multiply kT2's zero rows - content irrelevant, but
            # must be initialized for the race checker.
            nc.gpsimd.memset(qTC[t][64:128, :], 0.0)
        v_sb = [qkpool.tile([128, VW], bf16, tag=f"v{t}", name=f"v{t}")
                for t in range(T // TB)]
        # per-chunk normalized-O^T tiles (per-tile deps: deferred proj of
        # chunk t-1 must not wait on chunk t's normalize)
        prhs0 = [qkpool.tile([128, TQ], bf16, tag=f"prhs0{t}", name=f"prhs0{t}")
                 for t in range(NCH)]      # heads 0,1
        prhs1 = [qkpool.tile([128, TQ], bf16, tag=f"prhs1{t}", name=f"prhs1{t}")
                 for t in range(NCH)]      # head 2 (rows 64+ zero)
        for t in range(NCH):
            nc.gpsimd.memset(prhs1[t][64:128, :], 0.0)

        qT = [qTA, qTA, qTC]        # zero rows in kT select the head half
        kT = [kT0, kT1, kT2]

        # PSUM budget (8 banks): s 4 + po 2 + mix 2. "mix" is shared by
        # qkv-production psums and the projection psums (ring cycles in
        # program order).
        s_ps = ctx.enter_context(tc.tile_pool(name="s_ps", bufs=2, space="PSUM"))
        po_ps = ctx.enter_context(tc.tile_pool(name="po_ps", bufs=2, space="PSUM"))
        mix_ps = ctx.enter_context(tc.tile_pool(name="mix_ps", bufs=2, space="PSUM"))
        pt_p = ctx.enter_context(tc.tile_pool(name="pt_p", bufs=10))
        nrm = ctx.enter_context(tc.tile_pool(name="nrm", bufs=4))
        outp = ctx.enter_context(tc.tile_pool(name="outp", bufs=3))
        outp3 = ctx.enter_context(tc.tile_pool(name="outp3", bufs=6))

        def emit_qk_group(t, m):
            # chunk t of q^T/k^T; M-tiles: [q0|q1], [k0|k1], [q2|k2].
            # Generator: yields between matmuls so the filler can interleave
            # at single-matmul granularity.
            ps = mix_ps.tile([128, TQ], f32, tag="mix", name=f"ps_{t}_{m}")
            for j in range(NKT):
                nc.tensor.matmul(
                    ps[:],
                    wqk_sl(j, m),
                    xtile(t, j),
                    start=(j == 0),
                    stop=(j == NKT - 1 and not use_bias),
                )
                if j < NKT - 1:
                    yield
            if use_bias:
                nc.tensor.matmul(
                    ps[:], wqk_sl(6, m, rows=1),
                    xt1[:], start=False, stop=True,
                )
            if m == 0:
                nc.vector.tensor_copy(qTA[t][:], ps[:])                # q0;q1
            elif m == 1:
                nc.vector.tensor_copy(kT0[t][0:64, :], ps[0:64, :])    # k0
                nc.vector.tensor_copy(kT1[t][64:128, :], ps[64:128, :])  # k1
            else:
                nc.vector.tensor_copy(qTC[t][0:64, :], ps[0:64, :])    # q2
                nc.vector.tensor_copy(kT2[t][0:64, :], ps[64:128, :])  # k2
            yield

        def emit_v_group(t, tb):
            # v block tb in [t, d] layout; wv interleaves [v_h | ones] per
            # head. Without bias the ones columns are memset directly.
            psv = mix_ps.tile([128, TQ], f32, tag="mix", name=f"psv_{tb}")
            for j in range(NKT):
                nc.tensor.matmul(
                    psv[0:128, 0:VW],
                    xtile(t, j)[:, TB * (tb % NBL) : TB * (tb % NBL + 1)],
                    wv[:, VW * j : VW * (j + 1)],
                    start=(j == 0), stop=(j == NKT - 1 and not use_bias),
                )
                if j % 2 == 1:
                    yield
            if use_bias:
                nc.tensor.matmul(
                    psv[0:128, 0:VW],
                    xt1[0:1, 0:TB],
                    wv[0:1, VW * 6 : VW * 7],
                    start=False, stop=True,
                )
            nc.vector.tensor_copy(v_sb[tb][:], psv[:, 0:VW])
            if not use_bias:
                # ones columns on DVE (right behind the copy), NOT gpsimd:
                # the in-order gpsimd queue must stay clear for the
                # affine_selects that gate PV matmuls
                for h in range(H_LOC):
                    c1 = (D + 1) * h + D
                    nc.vector.memset(v_sb[tb][:, c1 : c1 + 1], 1.0)
            yield

        def emit_attn_chunk(i, filler):
            # Software-pipelined attention for one tq chunk, flattened over
            # (head, pair). The PE queue is in-order, so PV(p) emitted right
            # after S(p) would leave the queue head blocked on exp(p) for
            # ~1.2us while ready filler work sits uselessly behind it.
            # Instead each slot emits: filler pieces, S+exp of the NEXT
            # (head, pair), then PV of the current one - exp(p) completes
            # while the PE runs S(p+1) and fillers. The normalize broadcast
            # matmul is likewise deferred one slot so it never heads the PE
            # queue before its DVE-copied denominator is ready.
            nblk = NBL * (i + 1)
            npair = nblk // 2
            seq = [(h, p) for h in range(H_LOC) for p in range(npair)]
            po = {}
            pts = {}
            pending = []

            def emit_s_exp(h, p):
                # two tk-blocks share a [128, 1024] psum tile -> one exp
                ps2 = s_ps.tile([128, 2 * TQ], f32, tag="s", name=f"s_{i}_{h}_{p}")
                if SIMSAFE and p >= npair - 2:
                    # exp reads the gap between the two halves' written
                    # regions on diagonal pairs; zero it for the sim checker
                    nc.vector.memset(ps2[:], 0.0)
                c0s = []
                for half in range(2):
                    Bq = 2 * p + half
                    j = Bq - NBL * i
                    c0 = 0 if j < 0 else TB * j
                    c0s.append(c0)
                    off = TQ * half
                    nc.tensor.matmul(
                        ps2[:, off + c0 : off + TQ],
                        kT[h][Bq // NBL][:, TB * (Bq % NBL) : TB * (Bq % NBL + 1)],
                        qT[h][i][:, c0:TQ],
                        start=True, stop=True,
                    )
                pt = pt_p.tile([128, 2 * TQ], bf16, tag="pt", name=f"pt_{i}_{h}_{p}")
                nc.scalar.activation(pt[:, c0s[0] :], ps2[:, c0s[0] :], EXP)
                for half in range(2):
                    j = 2 * p + half - NBL * i
                    if j >= 0:
                        # causal: zero P^T where tk > tq (on idle GpSimd)
                        off = TQ * half
                        nc.gpsimd.affine_select(
                            pt[:, off + TB * j : off + TB * (j + 1)],
                            pt[:, off + TB * j : off + TB * (j + 1)],
                            pattern=[[1, TB]],
                            compare_op=mybir.AluOpType.is_ge,
                            fill=0.0,
                            base=0,
                            channel_multiplier=-1,
                        )
                pts[(h, p)] = (pt, c0s)

            def emit_pv(h, p):
                pt, c0s = pts.pop((h, p))
                vs = []
                for half in range(2):
                    Bq = 2 * p + half
                    c0 = c0s[half]
                    j = Bq - NBL * i
                    off = TQ * half
                    if 0 <= j < NBL - 1 and not (i == 0 and p == 0):
                        # diagonal block: the [c0, c0+TB) columns gate on the
                        # affine_select; emit the unmasked remainder first so
                        # the PE never waits on the GpSimd mask chain
                        nc.tensor.matmul(
                            po[h][:, c0 + TB : TQ],
                            v_sb[Bq][:, (D + 1) * h : (D + 1) * (h + 1)],
                            pt[:, off + c0 + TB : off + TQ],
                            start=(Bq == 0), stop=False,
                        )
                        vs.append((Bq, c0, c0 + TB, off))
                    else:
                        vs.append((Bq, c0, TQ, off))
                for Bq, c0, c1, off in vs:
                    nc.tensor.matmul(
                        po[h][:, c0:c1],
                        v_sb[Bq][:, (D + 1) * h : (D + 1) * (h + 1)],
                        pt[:, off + c0 : off + c1],
                        start=(Bq == 0), stop=(Bq == nblk - 1),
                    )

            def normalize_b(h):
                # pb matmul broadcasts the denominator to D rows; the fast
                # approx reciprocal and the multiply produce normalized O^T
                dst = (prhs0[i][64 * h : 64 * (h + 1), :]
                       if h < 2 else prhs1[i][0:64, :])
                d_sb = d_tiles[h]
                pb = mix_ps.tile([128, TQ], f32, tag="mix", name=f"pb_{i}_{h}")
                nc.tensor.matmul(pb[0:D, :], ones_b[:], d_sb[:],
                                 start=True, stop=True)
                rb = nrm.tile([D, TQ], f32, tag="rb", name=f"rb_{i}_{h}")
                nc.vector.reciprocal_approx_fast(rb[:], pb[0:D, :])
                nc.vector.tensor_mul(dst, po[h][0:D, :], rb[:])

            d_tiles = {}
            h0, p0 = seq[0]
            po[h0] = po_ps.tile([D + 1, TQ], f32, tag="po", name=f"po_{i}_{h0}")
            emit_s_exp(h0, p0)
            for idx, (h, p) in enumerate(seq):
                if idx + 1 < len(seq):
                    if filler is not None:
                        filler()
                    hn, pn = seq[idx + 1]
                    if pn == 0:
                        po[hn] = po_ps.tile([D + 1, TQ], f32, tag="po",
                                            name=f"po_{i}_{hn}")
                    emit_s_exp(hn, pn)
                emit_pv(h, p)
                if pending:
                    normalize_b(pending.pop(0))
                if p == npair - 1:
                    # denominator row -> SBUF on DVE now; the PE-side
                    # broadcast runs one slot later via `pending`
                    d_sb = nrm.tile([1, TQ], bf16, tag="d", name=f"d_{i}_{h}")
                    nc.vector.tensor_copy(d_sb[:], po[h][D : D + 1, :])
                    d_tiles[h] = d_sb
                    pending.append(h)
            while pending:
                normalize_b(pending.pop(0))

        osb_tiles = {}

        def emit_proj(i, n, fine_dma=False):
            # projection chunk (wp cols C..2C rows 64+ are zero; prhs1
            # zero-padded to K=128)
            pp = mix_ps.tile([128, TQ], f32, tag="mix", name=f"pp_{i}_{n}")
            nc.tensor.matmul(pp[:], wp[:, 128 * n : 128 * (n + 1)],
                             prhs0[i][:], start=True, stop=False)
            yield
            nc.tensor.matmul(pp[:], wp[:, C + 128 * n : C + 128 * (n + 1)],
                             prhs1[i][:], start=False, stop=True)
            if fine_dma:
                # last chunk: per-slice tiles + DMAs, copies alternating
                # DVE/Scalar and DMAs alternating Sync/Scalar queues (all
                # exps are done by now, so Scalar is free): the tail drains
                # on two engine+queue pairs in parallel
                osb = outp3.tile([128, TQ], bf16, tag="o3", name=f"osb3_{n}")
                if n % 2:
                    nc.scalar.copy(osb[:], pp[:])
                    nc.scalar.dma_start(out_d[i][:, TQ * n : TQ * (n + 1)], osb[:])
                else:
                    nc.vector.tensor_copy(osb[:], pp[:])
                    nc.sync.dma_start(out_d[i][:, TQ * n : TQ * (n + 1)], osb[:])
                yield
                return
            hf = n // 3
            if (i, hf) not in osb_tiles:
                osb_tiles[(i, hf)] = outp.tile([128, XH], bf16, tag="out",
                                               name=f"osb_{i}_{hf}")
            osb = osb_tiles[(i, hf)]
            dst = osb[:, TQ * (n % 3) : TQ * (n % 3 + 1)]
            if i == 0:
                # chunk 0's projection runs in the PE-rich, ACT-idle head of
                # the kernel: its psum->sbuf casts go on Scalar to keep DVE
                # clear for the qk/v production casts
                nc.scalar.copy(dst, pp[:])
            else:
                nc.vector.tensor_copy(dst, pp[:])
            if n % 3 == 2:
                nc.sync.dma_start(out_d[i][:, XH * hf : XH * (hf + 1)], osb[:])
            yield

        # Chunk-major pipeline. The attention inner loop is ACT(exp)-bound,
        # and the imbalance grows with the chunk index (chunk t has ~(t+1)
        # units of exp work but no production of its own to hide it behind).
        # So independent PE work is woven between attention pairs: chunks
        # 0-2 get chunk t+1's production; chunk 3 - the most exp-heavy
        # window - gets the deferred projections of chunks 1 and 2. Chunk
        # 0's projection runs right after its attention (early, while the
        # DMA queues are otherwise busy with inputs); chunk 3's forms the
        # tail with per-slice output DMAs.
        def drain(gen):
            for _ in gen:
                pass

        # Chunk-0 production is paced by the input DMA stream, so sweep
        # j-major across the m=0,1 groups (two open psums = mix pool size):
        # each arriving x/w piece feeds 2 matmuls instead of 1, and the m=2
        # re-sweep afterwards hits only resident tiles.
        ps01 = [mix_ps.tile([128, TQ], f32, tag="mix", name=f"ps_0_{m}")
                for m in range(2)]
        for j in range(NKT):
            for m in range(2):
                nc.tensor.matmul(
                    ps01[m][:], wqk_sl(j, m), xtile(0, j),
                    start=(j == 0), stop=(j == NKT - 1 and not use_bias),
                )
        if use_bias:
            for m in range(2):
                nc.tensor.matmul(ps01[m][:], wqk_sl(6, m, rows=1), xt1[:],
                                 start=False, stop=True)
        nc.vector.tensor_copy(qTA[0][:], ps01[0][:])
        nc.vector.tensor_copy(kT0[0][0:64, :], ps01[1][0:64, :])
        nc.vector.tensor_copy(kT1[0][64:128, :], ps01[1][64:128, :])
        drain(emit_qk_group(0, 2))
        for tb in range(NBL):
            drain(emit_v_group(0, tb))

        def run_piece(piece):
            kind, a, b = piece
            if kind == "qk":
                return emit_qk_group(a, b)
            elif kind == "v":
                return emit_v_group(a, b)
            return emit_proj(a, b)

        def make_stream(pieces):
            for piece in pieces:
                yield from run_piece(piece)

        fill_map = {
            t: [("qk", t + 1, m) for m in range(3)] +
               [("v", t + 1, tb) for tb in range(NBL * (t + 1), NBL * (t + 2))]
            for t in range(NCH - 1)
        }
        fill_map[NCH - 1] = [("proj", 1, n) for n in range(C // 128)] + \
                            [("proj", 2, n) for n in range(C // 128)]
        STEPS = {"qk": 6, "v": 4, "proj": 2}
        _SENT = object()

        for t in range(NCH):
            stream = make_stream(fill_map[t])
            nsteps = sum(STEPS[p[0]] for p in fill_map[t])
            nslots = H_LOC * (t + 1) * 2 - 1
            per_slot = -(-nsteps // nslots)

            def filler(stream=stream, per_slot=per_slot):
                for _ in range(per_slot):
                    if next(stream, _SENT) is _SENT:
                        break

            emit_attn_chunk(t, filler)
            drain(stream)
            if t == 0:
                for n in range(C // 128):
                    drain(emit_proj(0, n))
        for n in range(C // 128):
            drain(emit_proj(NCH - 1, n, fine_dma=True))

    nc.compile()
    return nc


_PROG_CACHE = {}


def kernel(x, Wqkv, bqkv, Wproj, bproj):
    global LAST_RESULT
    x = np.asarray(x, dtype=np.float32)
    Wqkv = np.asarray(Wqkv, dtype=np.float32)
    bqkv = np.asarray(bqkv, dtype=np.float32)
    Wproj = np.asarray(Wproj, dtype=np.float32)
    bproj = np.asarray(bproj, dtype=np.float32)

    Wq, Wk, Wv = Wqkv[:, 0:C], Wqkv[:, C : 2 * C], Wqkv[:, 2 * C : 3 * C]
    bq, bk, bv = bqkv[0:C], bqkv[C : 2 * C], bqkv[2 * C : 3 * C]
    scale = 1.0 / np.sqrt(D)

    use_bias = bool(np.any(bq) or np.any(bk) or np.any(bv))
    if use_bias not in _PROG_CACHE:
        _PROG_CACHE[use_bias] = _build_program(use_bias)
    nc = _PROG_CACHE[use_bias]

    in_maps = []
    for c in range(N_CORES):
        b = c // (N_CORES // B)
        g = c % (N_CORES // B)
        hs = slice(DL * g, DL * (g + 1))       # this core's head-dim rows/cols

        # x^T packed per chunk: [NCH, 128, NKT*TQ], k-tile-major columns
        xt = np.ascontiguousarray(
            x[b].T.reshape(NKT, 128, NCH, TQ).transpose(2, 1, 0, 3)
        ).reshape(NCH, 128, XW)
        xt1 = np.ones((1, TQ), np.float32)

        wq_loc = Wq[:, hs] * scale             # fold 1/sqrt(D) into q
        bq_loc = bq[hs] * scale
        wk_loc, bk_loc = Wk[:, hs], bk[hs]
        wv_loc, bv_loc = Wv[:, hs], bv[hs]

        wqk = np.zeros((C + 128, 2 * DL), np.float32)   # 7 k-tiles of 128
        wqk[0:C, 0:128] = wq_loc[:, 0:128]
        wqk[C, 0:128] = bq_loc[0:128]
        wqk[0:C, 128:256] = wk_loc[:, 0:128]
        wqk[C, 128:256] = bk_loc[0:128]
        wqk[0:C, 256:320] = wq_loc[:, 128:192]
        wqk[C, 256:320] = bq_loc[128:192]
        wqk[0:C, 320:384] = wk_loc[:, 128:192]
        wqk[C, 320:384] = bk_loc[128:192]
        wqk = np.ascontiguousarray(
            wqk.reshape(7, 128, 2 * DL).transpose(1, 0, 2)).reshape(128, 7 * 2 * DL)

        wv_pad = np.zeros((C + 128, VW), np.float32)
        for h in range(H_LOC):
            c0 = (D + 1) * h
            wv_pad[0:C, c0 : c0 + D] = wv_loc[:, D * h : D * (h + 1)]
            wv_pad[C, c0 : c0 + D] = bv_loc[D * h : D * (h + 1)]
            wv_pad[C, c0 + D] = 1.0            # ones column -> softmax denom
        wv_pad = np.ascontiguousarray(
            wv_pad.reshape(7, 128, VW).transpose(1, 0, 2)).reshape(128, 7 * VW)

        wp = np.zeros((128, 2 * C), np.float32)
        wp[:, 0:C] = Wproj[DL * g : DL * g + 128, :]
        wp[0:64, C : 2 * C] = Wproj[DL * g + 128 : DL * (g + 1), :]

        bf = ml_dtypes.bfloat16
        in_maps.append({"xt": xt.astype(bf), "xt1": xt1.astype(bf),
                        "wqk": wqk.astype(bf), "wv": wv_pad.astype(bf),
                        "wp": wp.astype(bf)})

    res = bass_utils.run_bass_kernel_spmd(nc, in_maps, core_ids=list(range(N_CORES)))
    LAST_RESULT = res

    out = np.zeros((B, T, C), np.float32)
    for c in range(N_CORES):
        b = c // (N_CORES // B)
        # outc [i, 128, n*512+col] -> [C, T] -> [T, C]
        outT = (res.results[c]["outc"].astype(np.float32)
                .reshape(NCH, 128, NKT, TQ).transpose(2, 1, 0, 3).reshape(C, T))
        out[b] += outT.T
    return out + bproj


if __name__ == "__main__":
    rng = np.random.default_rng(0)
    s = 1.0 / np.sqrt(C)
    ins = {
        "x": rng.standard_normal((B, T, C), dtype=np.float32),
        "Wqkv": rng.standard_normal((C, 3 * C), dtype=np.float32) * s,
        "bqkv": np.zeros(3 * C, np.float32),
        "Wproj": rng.standard_normal((C, C), dtype=np.float32) * s,
        "bproj": np.zeros(C, np.float32),
    }
    out = kernel(**ins)
    print("out", out.shape, out.dtype, float(np.abs(out).max()))


# revision 33
# speedup vs baseline: 1.0223x; 1.0223x over previous
"""Multi-head causal self-attention (B=2, T=2048, C=768, H=12, D=64) on 8
Trainium2 NeuronCores.

Sharding: 24 (batch, head) units -> 3 heads per core; cores 0-3 take batch 0,
cores 4-7 take batch 1. Each core computes q/k/v projections for its 3 heads,
flash-style causal attention fully on-chip (no T x T tensor ever touches HBM),
and a partial output projection with its 192-row slice of Wproj. The host sums
the 4 partial projections per batch.

Device design notes:
  - All matmuls bf16 with fp32 PSUM accumulation.
  - DRAM tensors are packed so every DMA row is 1.5-6KB contiguous (the DMA
    queue is packet-rate limited, so big packets are the difference between
    ~65 GB/s and ~350 GB/s effective). Weights stream on the Scalar engine's
    DMA queue, x and outputs on Sync's, so descriptor feeds run in parallel.
  - q^T/k^T live in [64, T]-per-head bf16 tiles; S^T = K^T.T @ Q^T is
    computed transposed [tk, tq] so exp(S^T) feeds the P.T @ V matmul
    directly - no on-chip transposes anywhere. K^T tiles are zero-padded to
    K=128 and the zero-block position selects which half of the shared
    [q0;q1] rhs tile contributes.
  - V is augmented with a ones column per head, so the PV accumulation
    yields the softmax denominator as psum row 64 for free.
  - Normalization: fast approx reciprocal of the denominator row ([1, TQ] on
    DVE straight from PSUM), partition-broadcast to 64 rows on GpSimd, one
    DVE multiply. No PE involvement.
  - Causal masking: matmul columns restricted to tq >= tk-block start; the
    diagonal 128x128 sub-block gets a strictly-lower-triangular zero mask on
    P^T (GpSimd affine_select) after exp. Chunk-major emission pipelines QKV
    production, attention, and the output projection.
  - Output is written per chunk-half in [128, 1536] layout (3KB rows);
    host reassembles and reduces.
"""

import os
import sys

sys.path.insert(0, "/opt/trn_rl_repo")

import ml_dtypes
import numpy as np

import concourse.bass as bass
import concourse.tile as tile
from concourse import bacc, mybir
from concourse import bass_utils

B, T, C = 2, 2048, 768
H, D = 12, 64
N_CORES = 8
H_LOC = 3           # heads per core
DL = H_LOC * D      # 192 local head dims
TQ = 512            # tq chunk (psum bank width)
TB = 128            # tk block
NCH = T // TQ       # 4 chunks
NBL = TQ // TB      # 4 blocks per chunk
NKT = C // 128      # 6 contraction k-tiles
VW = 196            # v psum width: 3*(D+1)=195 used + 1 pad
XW = NKT * TQ       # 3072 x cols per chunk
XH = XW // 2        # 1536 half-chunk cols

f32 = mybir.dt.float32
bf16 = mybir.dt.bfloat16
EXP = mybir.ActivationFunctionType.Exp
SIMSAFE = bool(os.environ.get("BASS_SIMSAFE"))  # zero psum holes for CoreSim

LAST_RESULT = None  # test harness reads exec_time_ns from here


def _build_program(use_bias: bool):
    from contextlib import ExitStack

    nc = bacc.Bacc("TRN2", target_bir_lowering=False, debug=False,
                   num_devices=N_CORES)

    xt_d = nc.dram_tensor("xt", [NCH, 128, XW], bf16, kind="ExternalInput").ap()
    xt1_d = nc.dram_tensor("xt1", [1, TQ], bf16, kind="ExternalInput").ap()
    wqk_d = nc.dram_tensor("wqk", [128, 7 * 2 * DL], bf16, kind="ExternalInput").ap()
    wv_d = nc.dram_tensor("wv", [128, 7 * VW], bf16, kind="ExternalInput").ap()
    wp_d = nc.dram_tensor("wp", [128, 2 * C], bf16, kind="ExternalInput").ap()
    out_d = nc.dram_tensor("outc", [NCH, 128, NKT * TQ], bf16,
                           kind="ExternalOutput").ap()

    with tile.TileContext(nc) as tc, ExitStack() as ctx:
        wpool = ctx.enter_context(tc.tile_pool(name="w", bufs=1))
        xpool = ctx.enter_context(tc.tile_pool(name="x", bufs=1))
        qkpool = ctx.enter_context(tc.tile_pool(name="qk", bufs=1))
        cpool = ctx.enter_context(tc.tile_pool(name="const", bufs=1))
        ones_b = cpool.tile([1, D], bf16)
        nc.vector.memset(ones_b[:], 1.0)
        ones3 = cpool.tile([1, 256], bf16)
        nc.vector.memset(ones3[:], 0.0)

        # --- input loads. Weights go on the Scalar engine's DMA queue, x on
        # Sync's: two descriptor feeds in parallel, all rows >= 1.5KB.
        # The first qk matmul needs only wqk k-tile 0 and x chunk-0 k-tile 0,
        # so those get their own small tiles/DMAs at the head of each queue;
        # the rest stream in bigger transfers behind them.
        weng = nc.scalar
        wqk0 = wpool.tile([128, 2 * DL], bf16, tag="wqk0", name="wqk0")
        weng.dma_start(wqk0[:], wqk_d[:, 0 : 2 * DL])
        wqk12 = wpool.tile([128, 2 * 2 * DL], bf16, tag="wqk12", name="wqk12")
        weng.dma_start(wqk12[:], wqk_d[:, 2 * DL : 3 * 2 * DL])
        wqkB = wpool.tile([128, 4 * 2 * DL], bf16, tag="wqkB", name="wqkB")
        weng.dma_start(wqkB[:], wqk_d[:, 3 * 2 * DL : 7 * 2 * DL])
        wv = wpool.tile([128, 7 * VW], bf16, tag="wv", name="wv")
        weng.dma_start(wv[:], wv_d[:])
        wp = wpool.tile([128, 2 * C], bf16, tag="wp", name="wp")
        weng.dma_start(wp[:], wp_d[:])
        if use_bias:
            xt1 = xpool.tile([1, TQ], bf16)
            weng.dma_start(xt1[:], xt1_d[:])

        xt0 = []
        for part, w in ((0, 1), (1, 2), (3, 3)):
            t_ = xpool.tile([128, TQ * w], bf16, tag=f"xt0_{part}",
                            name=f"xt0_{part}")
            nc.sync.dma_start(t_[:], xt_d[0][:, TQ * part : TQ * (part + w)])
            xt0.append(t_)
        xh = [None]
        for t in range(1, NCH):
            halves = []
            for hf in range(2):
                t_ = xpool.tile([128, XH], bf16, tag=f"xt{t}_{hf}",
                                name=f"xt{t}_{hf}")
                nc.sync.dma_start(t_[:], xt_d[t][:, XH * hf : XH * (hf + 1)])
                halves.append(t_)
            xh.append(halves)

        def xtile(t, j):
            if t == 0:
                part = 0 if j == 0 else (1 if j < 3 else 2)
                base = (0, 1, 3)[part]
                return xt0[part][:, TQ * (j - base) : TQ * (j - base + 1)]
            return xh[t][j // 3][:, TQ * (j % 3) : TQ * (j % 3 + 1)]

        def wqk_sl(j, m, rows=None):
            if j == 0:
                tl, base = wqk0, 0
            elif j < 3:
                tl, base = wqk12, (j - 1) * 2 * DL
            else:
                tl, base = wqkB, (j - 3) * 2 * DL
            sl = tl[:, base + 128 * m : base + 128 * (m + 1)]
            return sl if rows is None else tl[0:rows, base + 128 * m : base + 128 * (m + 1)]

        # Attention-stage tiles (bf16). S^T contraction is zero-padded to
        # K=128; each head's K^T has the other 64 rows zeroed, and the
        # zero-block position selects which half of the shared [q0;q1] rhs
        # tile contributes.
        qTA, qTC, kT0, kT1, kT2 = [], [], [], [], []
        for t in range(NCH):
            qTA.append(qkpool.tile([128, TQ], bf16, tag=f"qTA{t}", name=f"qTA{t}"))  # [q0 ; q1]
            qTC.append(qkpool.tile([128, TQ], bf16, tag=f"qTC{t}", name=f"qTC{t}"))  # [q2 ; *]
            kT0.append(qkpool.tile([128, TQ], bf16, tag=f"kT0{t}", name=f"kT0{t}"))  # [k0 ; 0]
            kT1.append(qkpool.tile([128, TQ], bf16, tag=f"kT1{t}", name=f"kT1{t}"))  # [0 ; k1]
            kT2.append(qkpool.tile([128, TQ], bf16, tag=f"kT2{t}", name=f"kT2{t}"))  # [k2 ; 0]
            nc.gpsimd.memset(kT0[t][64:128, :], 0.0)
            nc.gpsimd.memset(kT1[t][0:64, :], 0.0)
            nc.gpsimd.memset(kT2[t][64:128, :], 0.0)
            # qTC rows 64+ multiply kT2's zero rows - content irrelevant, but
            # must be initialized for the race checker.
            nc.gpsimd.memset(qTC[t][64:128, :], 0.0)
        v_sb = [qkpool.tile([128, VW], bf16, tag=f"v{t}", name=f"v{t}")
                for t in range(T // TB)]
        # per-chunk normalized-O^T tiles (per-tile deps: deferred proj of
        # chunk t-1 must not wait on chunk t's normalize)
        prhs0 = [qkpool.tile([128, TQ], bf16, tag=f"prhs0{t}", name=f"prhs0{t}")
                 for t in range(NCH)]      # heads 0,1
        prhs1 = [qkpool.tile([128, TQ], bf16, tag=f"prhs1{t}", name=f"prhs1{t}")
                 for t in range(NCH)]      # head 2 (rows 64+ zero)
        for t in range(NCH):
            nc.gpsimd.memset(prhs1[t][64:128, :], 0.0)

        qT = [qTA, qTA, qTC]        # zero rows in kT select the head half
        kT = [kT0, kT1, kT2]

        # PSUM budget (8 banks): s 4 + po 2 + mix 2. "mix" is shared by
        # qkv-production psums and the projection psums (ring cycles in
        # program order).
        s_ps = ctx.enter_context(tc.tile_pool(name="s_ps", bufs=2, space="PSUM"))
        po_ps = ctx.enter_context(tc.tile_pool(name="po_ps", bufs=2, space="PSUM"))
        mix_ps = ctx.enter_context(tc.tile_pool(name="mix_ps", bufs=2, space="PSUM"))
        pt_p = ctx.enter_context(tc.tile_pool(name="pt_p", bufs=10))
        nrm = ctx.enter_context(tc.tile_pool(name="nrm", bufs=4))
        outp = ctx.enter_context(tc.tile_pool(name="outp", bufs=3))
        outp3 = ctx.enter_context(tc.tile_pool(name="outp3", bufs=6))

        def emit_qk_group(t, m):
            # chunk t of q^T/k^T; M-tiles: [q0|q1], [k0|k1], [q2|k2].
            # Generator: yields between matmuls so the filler can interleave
            # at single-matmul granularity.
            ps = mix_ps.tile([128, TQ], f32, tag="mix", name=f"ps_{t}_{m}")
            for j in range(NKT):
                nc.tensor.matmul(
                    ps[:],
                    wqk_sl(j, m),
                    xtile(t, j),
                    start=(j == 0),
                    stop=(j == NKT - 1 and not use_bias),
                )
                if j < NKT - 1:
                    yield
            if use_bias:
                nc.tensor.matmul(
                    ps[:], wqk_sl(6, m, rows=1),
                    xt1[:], start=False, stop=True,
                )
            if m == 0:
                nc.vector.tensor_copy(qTA[t][:], ps[:])                # q0;q1
            elif m == 1:
                nc.vector.tensor_copy(kT0[t][0:64, :], ps[0:64, :])    # k0
                nc.vector.tensor_copy(kT1[t][64:128, :], ps[64:128, :])  # k1
            else:
                nc.vector.tensor_copy(qTC[t][0:64, :], ps[0:64, :])    # q2
                nc.vector.tensor_copy(kT2[t][0:64, :], ps[64:128, :])  # k2
            yield

        def emit_v_group(t, tb):
            # v block tb in [t, d] layout; wv interleaves [v_h | ones] per
            # head. Without bias the ones columns are memset directly.
            psv = mix_ps.tile([128, TQ], f32, tag="mix", name=f"psv_{tb}")
            for j in range(NKT):
                nc.tensor.matmul(
                    psv[0:128, 0:VW],
                    xtile(t, j)[:, TB * (tb % NBL) : TB * (tb % NBL + 1)],
                    wv[:, VW * j : VW * (j + 1)],
                    start=(j == 0), stop=(j == NKT - 1 and not use_bias),
                )
                if j % 2 == 1:
                    yield
            if use_bias:
                nc.tensor.matmul(
                    psv[0:128, 0:VW],
                    xt1[0:1, 0:TB],
                    wv[0:1, VW * 6 : VW * 7],
                    start=False, stop=True,
                )
            nc.vector.tensor_copy(v_sb[tb][:], psv[:, 0:VW])
            if not use_bias:
                # ones columns on DVE (right behind the copy), NOT gpsimd:
                # the in-order gpsimd queue must stay clear for the
                # affine_selects that gate PV matmuls
                for h in range(H_LOC):
                    c1 = (D + 1) * h + D
                    nc.vector.memset(v_sb[tb][:, c1 : c1 + 1], 1.0)
            yield

        def emit_attn_chunk(i, filler):
            # Software-pipelined attention for one tq chunk, flattened over
            # (head, pair). The PE queue is in-order, so PV(p) emitted right
            # after S(p) would leave the queue head blocked on exp(p) for
            # ~1.2us while ready filler work sits uselessly behind it.
            # Instead each slot emits: filler pieces, S+exp of the NEXT
            # (head, pair), then PV of the current one - exp(p) completes
            # while the PE runs S(p+1) and fillers. The normalize broadcast
            # matmul is likewise deferred one slot so it never heads the PE
            # queue before its DVE-copied denominator is ready.
            nblk = NBL * (i + 1)
            npair = nblk // 2
            seq = [(h, p) for h in range(H_LOC) for p in range(npair)]
            po = {}
            pts = {}
            pending = []

            def emit_s_exp(h, p):
                # two tk-blocks share a [128, 1024] psum tile -> one exp
                ps2 = s_ps.tile([128, 2 * TQ], f32, tag="s", name=f"s_{i}_{h}_{p}")
                if SIMSAFE and p >= npair - 2:
                    # exp reads the gap between the two halves' written
                    # regions on diagonal pairs; zero it for the sim checker
                    nc.vector.memset(ps2[:], 0.0)
                c0s = []
                for half in range(2):
                    Bq = 2 * p + half
                    j = Bq - NBL * i
                    c0 = 0 if j < 0 else TB * j
                    c0s.append(c0)
                    off = TQ * half
                    nc.tensor.matmul(
                        ps2[:, off + c0 : off + TQ],
                        kT[h][Bq // NBL][:, TB * (Bq % NBL) : TB * (Bq % NBL + 1)],
                        qT[h][i][:, c0:TQ],
                        start=True, stop=True,
                    )
                pt = pt_p.tile([128, 2 * TQ], bf16, tag="pt", name=f"pt_{i}_{h}_{p}")
                nc.scalar.activation(pt[:, c0s[0] :], ps2[:, c0s[0] :], EXP)
                for half in range(2):
                    j = 2 * p + half - NBL * i
                    if j >= 0:
                        # causal: zero P^T where tk > tq (on idle GpSimd)
                        off = TQ * half
                        nc.gpsimd.affine_select(
                            pt[:, off + TB * j : off + TB * (j + 1)],
                            pt[:, off + TB * j : off + TB * (j + 1)],
                            pattern=[[1, TB]],
                            compare_op=mybir.AluOpType.is_ge,
                            fill=0.0,
                            base=0,
                            channel_multiplier=-1,
                        )
                pts[(h, p)] = (pt, c0s)

            def emit_pv(h, p):
                pt, c0s = pts.pop((h, p))
                for half in range(2):
                    Bq = 2 * p + half
                    c0 = c0s[half]
                    nc.tensor.matmul(
                        po[h][:, c0:TQ],
                        v_sb[Bq][:, (D + 1) * h : (D + 1) * (h + 1)],
                        pt[:, TQ * half + c0 : TQ * half + TQ],
                        start=(Bq == 0), stop=(Bq == nblk - 1),
                    )

            def normalize_b(h):
                # pb matmul broadcasts the denominator to D rows; the fast
                # approx reciprocal and the multiply produce normalized O^T
                dst = (prhs0[i][64 * h : 64 * (h + 1), :]
                       if h < 2 else prhs1[i][0:64, :])
                d_sb = d_tiles[h]
                pb = mix_ps.tile([128, TQ], f32, tag="mix", name=f"pb_{i}_{h}")
                nc.tensor.matmul(pb[0:D, :], ones_b[:], d_sb[:],
                                 start=True, stop=True)
                rb = nrm.tile([D, TQ], f32, tag="rb", name=f"rb_{i}_{h}")
                nc.vector.reciprocal_approx_fast(rb[:], pb[0:D, :])
                nc.vector.tensor_mul(dst, po[h][0:D, :], rb[:])

            d_tiles = {}
            h0, p0 = seq[0]
            po[h0] = po_ps.tile([D + 1, TQ], f32, tag="po", name=f"po_{i}_{h0}")
            emit_s_exp(h0, p0)
            for idx, (h, p) in enumerate(seq):
                if idx + 1 < len(seq):
                    if filler is not None:
                        filler()
                    hn, pn = seq[idx + 1]
                    if pn == 0:
                        po[hn] = po_ps.tile([D + 1, TQ], f32, tag="po",
                                            name=f"po_{i}_{hn}")
                    emit_s_exp(hn, pn)
                emit_pv(h, p)
                for k in range(len(pending)):
                    pending[k][1] -= 1
                while pending and pending[0][1] <= 0:
                    normalize_b(pending.pop(0)[0])
                if p == npair - 1:
                    # denominator row -> SBUF on DVE now; the PE-side
                    # broadcast runs two slots later via `pending`, by which
                    # time the DVE queue has surely drained the copy
                    d_sb = nrm.tile([1, TQ], bf16, tag="d", name=f"d_{i}_{h}")
                    nc.vector.tensor_copy(d_sb[:], po[h][D : D + 1, :])
                    d_tiles[h] = d_sb
                    pending.append([h, 2 if npair > 2 else 1])
            while pending:
                normalize_b(pending.pop(0)[0])

        osb_tiles = {}

        def emit_proj(i, n, fine_dma=False):
            # projection chunk (wp cols C..2C rows 64+ are zero; prhs1
            # zero-padded to K=128)
            pp = mix_ps.tile([128, TQ], f32, tag="mix", name=f"pp_{i}_{n}")
            nc.tensor.matmul(pp[:], wp[:, 128 * n : 128 * (n + 1)],
                             prhs0[i][:], start=True, stop=False)
            yield
            nc.tensor.matmul(pp[:], wp[:, C + 128 * n : C + 128 * (n + 1)],
                             prhs1[i][:], start=False, stop=True)
            if fine_dma:
                # last chunk: per-slice tiles + DMAs, copies alternating
                # DVE/Scalar and DMAs alternating Sync/Scalar queues (all
                # exps are done by now, so Scalar is free): the tail drains
                # on two engine+queue pairs in parallel
                osb = outp3.tile([128, TQ], bf16, tag="o3", name=f"osb3_{n}")
                if n % 2:
                    nc.scalar.copy(osb[:], pp[:])
                    nc.scalar.dma_start(out_d[i][:, TQ * n : TQ * (n + 1)], osb[:])
                else:
                    nc.vector.tensor_copy(osb[:], pp[:])
                    nc.sync.dma_start(out_d[i][:, TQ * n : TQ * (n + 1)], osb[:])
                yield
                return
            hf = n // 3
            if (i, hf) not in osb_tiles:
                osb_tiles[(i, hf)] = outp.tile([128, XH], bf16, tag="out",
                                               name=f"osb_{i}_{hf}")
            osb = osb_tiles[(i, hf)]
            dst = osb[:, TQ * (n % 3) : TQ * (n % 3 + 1)]
            if i == 0:
                # chunk 0's projection runs in the PE-rich, ACT-idle head of
                # the kernel: its psum->sbuf casts go on Scalar to keep DVE
                # clear for the qk/v production casts
                nc.scalar.copy(dst, pp[:])
            else:
                nc.vector.tensor_copy(dst, pp[:])
            if n % 3 == 2:
                nc.sync.dma_start(out_d[i][:, XH * hf : XH * (hf + 1)], osb[:])
            yield

        # Chunk-major pipeline. The attention inner loop is ACT(exp)-bound,
        # and the imbalance grows with the chunk index (chunk t has ~(t+1)
        # units of exp work but no production of its own to hide it behind).
        # So independent PE work is woven between attention pairs: chunks
        # 0-2 get chunk t+1's production; chunk 3 - the most exp-heavy
        # window - gets the deferred projections of chunks 1 and 2. Chunk
        # 0's projection runs right after its attention (early, while the
        # DMA queues are otherwise busy with inputs); chunk 3's forms the
        # tail with per-slice output DMAs.
        def drain(gen):
            for _ in gen:
                pass

        # Warm up the PE clock (p-state ramps 0.65->1.2->2.4GHz over ~3us of
        # continuous execution) with dependency-free dummy matmuls while the
        # first input DMAs stream in.
        wps = mix_ps.tile([128, TQ], f32, tag="mix", name="warm")
        for w in range(8):
            nc.tensor.matmul(wps[0:D, 0:256], ones_b[:], ones3[0:1, 0:256],
                             start=True, stop=True)
        for m in range(3):
            drain(emit_qk_group(0, m))
        for tb in range(NBL):
            drain(emit_v_group(0, tb))

        def run_piece(piece):
            kind, a, b = piece
            if kind == "qk":
                return emit_qk_group(a, b)
            elif kind == "v":
                return emit_v_group(a, b)
            return emit_proj(a, b)

        def make_stream(pieces):
            for piece in pieces:
                yield from run_piece(piece)

        fill_map = {
            t: [("qk", t + 1, m) for m in range(3)] +
               [("v", t + 1, tb) for tb in range(NBL * (t + 1), NBL * (t + 2))]
            for t in range(NCH - 1)
        }
        fill_map[NCH - 1] = [("proj", 1, n) for n in range(C // 128)] + \
                            [("proj", 2, n) for n in range(C // 128)]
        STEPS = {"qk": 6, "v": 4, "proj": 2}
        _SENT = object()

        for t in range(NCH):
            stream = make_stream(fill_map[t])
            nsteps = sum(STEPS[p[0]] for p in fill_map[t])
            nslots = H_LOC * (t + 1) * 2 - 1
            per_slot = -(-nsteps // nslots)

            def filler(stream=stream, per_slot=per_slot):
                for _ in range(per_slot):
                    if next(stream, _SENT) is _SENT:
                        break

            emit_attn_chunk(t, filler)
            drain(stream)
            if t == 0:
                for n in range(C // 128):
                    drain(emit_proj(0, n))
        for n in range(C // 128):
            drain(emit_proj(NCH - 1, n, fine_dma=True))

    nc.compile()
    return nc


_PROG_CACHE = {}


def kernel(x, Wqkv, bqkv, Wproj, bproj):
    global LAST_RESULT
    x = np.asarray(x, dtype=np.float32)
    Wqkv = np.asarray(Wqkv, dtype=np.float32)
    bqkv = np.asarray(bqkv, dtype=np.float32)
    Wproj = np.asarray(Wproj, dtype=np.float32)
    bproj = np.asarray(bproj, dtype=np.float32)

    Wq, Wk, Wv = Wqkv[:, 0:C], Wqkv[:, C : 2 * C], Wqkv[:, 2 * C : 3 * C]
    bq, bk, bv = bqkv[0:C], bqkv[C : 2 * C], bqkv[2 * C : 3 * C]
    scale = 1.0 / np.sqrt(D)

    use_bias = bool(np.any(bq) or np.any(bk) or np.any(bv))
    if use_bias not in _PROG_CACHE:
        _PROG_CACHE[use_bias] = _build_program(use_bias)
    nc = _PROG_CACHE[use_bias]

    in_maps = []
    for c in range(N_CORES):
        b = c // (N_CORES // B)
        g = c % (N_CORES // B)
        hs = slice(DL * g, DL * (g + 1))       # this core's head-dim rows/cols

        # x^T packed per chunk: [NCH, 128, NKT*TQ], k-tile-major columns
        xt = np.ascontiguousarray(
            x[b].T.reshape(NKT, 128, NCH, TQ).transpose(2, 1, 0, 3)
        ).reshape(NCH, 128, XW)
        xt1 = np.ones((1, TQ), np.float32)

        wq_loc = Wq[:, hs] * scale             # fold 1/sqrt(D) into q
        bq_loc = bq[hs] * scale
        wk_loc, bk_loc = Wk[:, hs], bk[hs]
        wv_loc, bv_loc = Wv[:, hs], bv[hs]

        wqk = np.zeros((C + 128, 2 * DL), np.float32)   # 7 k-tiles of 128
        wqk[0:C, 0:128] = wq_loc[:, 0:128]
        wqk[C, 0:128] = bq_loc[0:128]
        wqk[0:C, 128:256] = wk_loc[:, 0:128]
        wqk[C, 128:256] = bk_loc[0:128]
        wqk[0:C, 256:320] = wq_loc[:, 128:192]
        wqk[C, 256:320] = bq_loc[128:192]
        wqk[0:C, 320:384] = wk_loc[:, 128:192]
        wqk[C, 320:384] = bk_loc[128:192]
        wqk = np.ascontiguousarray(
            wqk.reshape(7, 128, 2 * DL).transpose(1, 0, 2)).reshape(128, 7 * 2 * DL)

        wv_pad = np.zeros((C + 128, VW), np.float32)
        for h in range(H_LOC):
            c0 = (D + 1) * h
            wv_pad[0:C, c0 : c0 + D] = wv_loc[:, D * h : D * (h + 1)]
            wv_pad[C, c0 : c0 + D] = bv_loc[D * h : D * (h + 1)]
            wv_pad[C, c0 + D] = 1.0            # ones column -> softmax denom
        wv_pad = np.ascontiguousarray(
            wv_pad.reshape(7, 128, VW).transpose(1, 0, 2)).reshape(128, 7 * VW)

        wp = np.zeros((128, 2 * C), np.float32)
        wp[:, 0:C] = Wproj[DL * g : DL * g + 128, :]
        wp[0:64, C : 2 * C] = Wproj[DL * g + 128 : DL * (g + 1), :]

        bf = ml_dtypes.bfloat16
        in_maps.append({"xt": xt.astype(bf), "xt1": xt1.astype(bf),
                        "wqk": wqk.astype(bf), "wv": wv_pad.astype(bf),
                        "wp": wp.astype(bf)})

    res = bass_utils.run_bass_kernel_spmd(nc, in_maps, core_ids=list(range(N_CORES)))
    LAST_RESULT = res

    out = np.zeros((B, T, C), np.float32)
    for c in range(N_CORES):
        b = c // (N_CORES // B)
        # outc [i, 128, n*512+col] -> [C, T] -> [T, C]
        outT = (res.results[c]["outc"].astype(np.float32)
                .reshape(NCH, 128, NKT, TQ).transpose(2, 1, 0, 3).reshape(C, T))
        out[b] += outT.T
    return out + bproj


if __name__ == "__main__":
    rng = np.random.default_rng(0)
    s = 1.0 / np.sqrt(C)
    ins = {
        "x": rng.standard_normal((B, T, C), dtype=np.float32),
        "Wqkv": rng.standard_normal((C, 3 * C), dtype=np.float32) * s,
        "bqkv": np.zeros(3 * C, np.float32),
        "Wproj": rng.standard_normal((C, C), dtype=np.float32) * s,
        "bproj": np.zeros(C, np.float32),
    }
    out = kernel(**ins)
    print("out", out.shape, out.dtype, float(np.abs(out).max()))
